# revision 12
# baseline (speedup 1.0000x reference)
"""Trainium2 Bass kernel for the pre-norm attention + SwiGLU FFN layer.

Sharding: tokens (batch*seq flattened) split across 8 cores — 512 tokens
each; cores 0-3 hold batch 0, cores 4-7 batch 1. All per-token work (LNs,
projections, rope, FFN) is fully local with replicated weights; attention
gathers the rope'd K and ones-padded V across each 4-core batch group with
one AllGather, then each core attends its 512 queries over the full 2048
context. The ones column appended to V makes the PV matmul emit softmax
denominators for free (row 64 of each head's PV output); softmax skips max
subtraction (scores are O(1) after QK-norm).

Weights are transposed host-side (numpy) so every matmul operand has the
contraction dim on partitions; matmuls run as float32r (full PE rate at
free-dim >= 256). rope cos/sin are host-expanded to per-token [T, D] tables
with the rotation sign folded in, so on-device rope is 2 strided copies +
3 elementwise ops.
"""

import os
import tempfile

import numpy as np

import bass_rust
import concourse.bass as bass
import concourse.mybir as mybir
import concourse.tile as tile
from concourse.bass_utils import run_bass_kernel_spmd
from concourse.masks import make_identity
from concourse.vector_clock import ScopedClock

F32 = mybir.dt.float32
F32R = mybir.dt.float32r
AF = mybir.ActivationFunctionType

N_CORES = 8
GROUP = 4
EPS = 1e-6

# ---------------------------------------------------------------------------
# Workaround for this walrus build's 1-wait-per-instruction encoding limit.
# ---------------------------------------------------------------------------
_MAX_WAITS = 1
_carrier_id = [0]


def _patched_drain_and_barrier(self, tick_clock, wait_clock):
    nc = self.nc
    drain_inst = nc.sync.drain()
    wait_clock.add_sem_waits(
        drain_inst.ins, ScopedClock({None: tick_clock.global_clock})
    )
    si = drain_inst.ins.sync_info
    waits = list(si.on_wait)
    if len(waits) > _MAX_WAITS:
        drain_inst.ins.sync_info = bass_rust.SyncInfo(
            on_wait=waits[:_MAX_WAITS], on_update=list(si.on_update)
        )
        rest = waits[_MAX_WAITS:]
        while rest:
            chunk, rest = rest[:_MAX_WAITS], rest[_MAX_WAITS:]
            extra = nc.sync.drain()
            extra.ins.sync_info = bass_rust.SyncInfo(on_wait=chunk, on_update=[])

    nc.all_engine_barrier()
    assert self.sems is not None
    popped = nc._tile_sem_poison_stack.pop()
    assert popped is self._sem_poison
    nc.clear_and_free_semaphores(list(self.sems.allocated().values()))
    nc.all_engine_barrier()


tile.TileContext._drain_and_barrier = _patched_drain_and_barrier


def _split_all_waits(nc, max_waits=_MAX_WAITS):
    for fn in nc.m.functions:
        for bb in fn.blocks:
            insts = list(bb.instructions)
            out = []
            changed = False
            for inst in insts:
                si = getattr(inst, "sync_info", None)
                if si is not None and si.on_wait and len(si.on_wait) > max_waits:
                    waits = list(si.on_wait)
                    updates = list(si.on_update)
                    extra, keep = waits[:-max_waits], waits[-max_waits:]
                    while extra:
                        chunk, extra = extra[:max_waits], extra[max_waits:]
                        _carrier_id[0] += 1
                        nop = mybir.InstNoOp(name=f"I-waitcar-{_carrier_id[0]}")
                        nop.engine = inst.engine
                        nop.sync_info = bass_rust.SyncInfo(on_wait=chunk, on_update=[])
                        nc.register_instruction(nop)
                        out.append(nop)
                    inst.sync_info = bass_rust.SyncInfo(on_wait=keep, on_update=updates)
                    changed = True
                out.append(inst)
            if changed:
                bb.instructions = out


# ---------------------------------------------------------------------------
# Graph builder (one SPMD program for all 8 cores)
# ---------------------------------------------------------------------------

def build_nc(T=512, D=1024, H=16, HD=64, FFN=4096, flags=frozenset()):
    """T: tokens per core; context = GROUP*T. flags: subset of
    {ln1_gb, qn_gb, kn_gb, ln2_gb, bqkv, bout, b1, b2, b3}."""
    NT = T // 128            # token tiles per core
    ND = D // 128            # model-dim tiles
    NH = FFN // 128          # ffn hidden tiles
    HP = H // 2              # head pairs (= ND)
    D3 = 3 * D
    NCH = D3 // 512          # qkv output chunks of 512
    VW = H * (HD + 1)        # padded v width per token (1040)
    KVF = D * T + T * VW     # floats in the per-core kv bounce

    nc = bass.Bass(trn_type="TRN2", num_devices=N_CORES)

    x_p = nc.declare_dram_parameter("x", [T, D], F32, isOutput=False)
    cos_p = nc.declare_dram_parameter("cosfull", [T, D], F32, isOutput=False)
    sin_p = nc.declare_dram_parameter("sinmod", [T, D], F32, isOutput=False)
    wqkv_p = nc.declare_dram_parameter("wqkvT", [D, D3], F32, isOutput=False)
    wout_p = nc.declare_dram_parameter("woutT", [D, D], F32, isOutput=False)
    w1_p = nc.declare_dram_parameter("w1T", [D, FFN], F32, isOutput=False)
    w3_p = nc.declare_dram_parameter("w3T", [D, FFN], F32, isOutput=False)
    w2_p = nc.declare_dram_parameter("w2T", [FFN, D], F32, isOutput=False)
    vecs = {}
    for name, size in [("ln1_g", D), ("ln1_b", D), ("qn_g", D), ("qn_b", D),
                       ("kn_g", D), ("kn_b", D), ("ln2_g", D), ("ln2_b", D),
                       ("b_qkv", D3), ("b_out", D), ("b1", FFN), ("b3", FFN),
                       ("b2", D)]:
        flag = {"ln1_g": "ln1_gb", "ln1_b": "ln1_gb", "qn_g": "qn_gb",
                "qn_b": "qn_gb", "kn_g": "kn_gb", "kn_b": "kn_gb",
                "ln2_g": "ln2_gb", "ln2_b": "ln2_gb", "b_qkv": "bqkv",
                "b_out": "bout", "b1": "b1", "b3": "b3", "b2": "b2"}[name]
        if flag in flags:
            vecs[name] = nc.declare_dram_parameter(name, [size], F32, isOutput=False)
    out_p = nc.declare_dram_parameter("out", [T, D], F32, isOutput=True)

    kv_in = nc.dram_tensor("kv_in", [KVF], F32)
    kv_all = nc.dram_tensor("kv_all", [GROUP * KVF], F32)

    def bcast_ap(param, width):
        return bass.AP(tensor=param.ap().tensor, offset=0,
                       ap=[[0, 128], [1, width]])

    from contextlib import ExitStack
    with tile.TileContext(nc) as tc, ExitStack() as stack:
        const = stack.enter_context(tc.tile_pool(name="const", bufs=1))
        ident = const.tile([128, 128], F32, tag="ident")
        make_identity(nc, ident)
        sel = const.tile([65, 128], F32, tag="sel")
        nc.vector.memset(sel, 0.0)
        nc.vector.memset(sel[64:65, :], 1.0)
        eps_t = const.tile([128, 1], F32, tag="eps")
        nc.vector.memset(eps_t, EPS)

        bc_tiles = {}
        for name in ("ln1_g", "ln1_b", "qn_g", "qn_b", "kn_g", "kn_b",
                     "ln2_g", "ln2_b", "b_out", "b2"):
            if name in vecs:
                t = const.tile([128, D], F32, tag=f"bc_{name}")
                nc.sync.dma_start(out=t, in_=bcast_ap(vecs[name], D))
                bc_tiles[name] = t
        if "b_qkv" in vecs:
            t = const.tile([128, D3], F32, tag="bc_bqkv")
            nc.sync.dma_start(out=t, in_=bcast_ap(vecs["b_qkv"], D3))
            bc_tiles["b_qkv"] = t
        for name in ("b1", "b3"):
            if name in vecs:
                # per-hidden scalars: [128, NH] with element (p, ht) = b[ht*128+p]
                t = const.tile([128, NH], F32, tag=f"col_{name}")
                ap = bass.AP(tensor=vecs[name].ap().tensor, offset=0,
                             ap=[[1, 128], [128, NH]])
                nc.sync.dma_start(out=t, in_=ap)
                bc_tiles[name] = t

        stat = stack.enter_context(tc.tile_pool(name="stat", bufs=4))
        xres = stack.enter_context(tc.tile_pool(name="xres", bufs=1))
        o1p = stack.enter_context(tc.tile_pool(name="o1p", bufs=1))

        x_N = [xres.tile([128, D], F32, tag=f"x{t}", name=f"x{t}") for t in range(NT)]
        out1_N = [o1p.tile([128, D], F32, tag=f"o1{t}", name=f"o1{t}") for t in range(NT)]

        def layer_norm_tiles(src_tile, dst_tile, gname):
            """dst = LN(src) with optional gain/bias, both [128, D]."""
            st = stat.tile([128, 2, 6], F32, tag="lnst")
            nc.vector.bn_stats(out=st[:, 0, :], in_=src_tile[:, 0:512])
            nc.vector.bn_stats(out=st[:, 1, :], in_=src_tile[:, 512:1024])
            mv = stat.tile([128, 2], F32, tag="lnmv")
            nc.vector.bn_aggr(out=mv, in_=st)
            rstd = stat.tile([128, 1], F32, tag="lnrstd")
            nc.scalar.activation(out=rstd, in_=mv[:, 1:2], func=AF.Sqrt,
                                 bias=eps_t, scale=1.0, alpha=0.0)
            nc.vector.reciprocal(out=rstd, in_=rstd)
            negmr = stat.tile([128, 1], F32, tag="lnnm")
            nc.vector.tensor_mul(out=negmr, in0=mv[:, 0:1], in1=rstd)
            nc.scalar.mul(out=negmr, in_=negmr, mul=-1.0)
            nc.scalar.activation(out=dst_tile, in_=src_tile, func=AF.Identity,
                                 scale=rstd, bias=negmr, alpha=0.0)
            if f"{gname}_g" in bc_tiles:
                nc.vector.tensor_mul(out=dst_tile, in0=dst_tile,
                                     in1=bc_tiles[f"{gname}_g"])
                nc.vector.tensor_add(out=dst_tile, in0=dst_tile,
                                     in1=bc_tiles[f"{gname}_b"])

        # ---- Phase A: load x, LN1, transpose h -> h_T --------------------
        qkv_res_cm = tc.tile_pool(name="qkv_res", bufs=1)
        qkv_res = qkv_res_cm.__enter__()
        q_T = [qkv_res.tile([128, T], F32, tag=f"qT{d}", name=f"qT{d}")
               for d in range(ND)]
        k_T = [qkv_res.tile([128, T], F32, tag=f"kT{d}", name=f"kT{d}")
               for d in range(ND)]
        v_pad = [qkv_res.tile([128, H, HD + 1], F32, tag=f"vp{t}", name=f"vp{t}")
                 for t in range(NT)]
        qknp_cm = tc.tile_pool(name="qknp", bufs=1)
        qknp = qknp_cm.__enter__()
        q_N = [qknp.tile([128, D], F32, tag=f"qN{t}", name=f"qN{t}") for t in range(NT)]
        k_N = [qknp.tile([128, D], F32, tag=f"kN{t}", name=f"kN{t}") for t in range(NT)]
        hTpool_cm = tc.tile_pool(name="hTpool", bufs=1)
        hTpool = hTpool_cm.__enter__()
        h_T = [hTpool.tile([128, T], F32, tag=f"hT{d}", name=f"hT{d}")
               for d in range(ND)]
        with (
            tc.tile_pool(name="hpool", bufs=2) as hpool,
            tc.tile_pool(name="trps", bufs=4, space="PSUM") as trps,
        ):
            for t in range(NT):
                nc.sync.dma_start(out=x_N[t], in_=x_p.ap()[t * 128:(t + 1) * 128, :])
                h_N = hpool.tile([128, D], F32, tag="hN")
                layer_norm_tiles(x_N[t], h_N, "ln1")
                for d in range(ND):
                    ptr = trps.tile([128, 128], F32, tag="trp")
                    nc.tensor.transpose(ptr, h_N[:, d * 128:(d + 1) * 128], ident)
                    nc.vector.tensor_copy(
                        out=h_T[d][:, t * 128:(t + 1) * 128].bitcast(F32R), in_=ptr)

        # ---- Phase B: QKV projection (h_T stationary, wT moving) ---------
        for t in range(NT):
            nc.vector.memset(v_pad[t][:, :, HD:HD + 1], 1.0)

        with (
            tc.tile_pool(name="wq", bufs=3) as wq,
            tc.tile_pool(name="mmps", bufs=8, space="PSUM") as mmps,
        ):
            for ch in range(NCH):
                ps = [mmps.tile([128, 512], F32, tag="qkvps", name=f"qkvps_{ch}_{t}") for t in range(NT)]
                for d in range(ND):
                    w = wq.tile([128, 512], F32, tag="wqt")
                    nc.sync.dma_start(
                        out=w.bitcast(F32R),
                        in_=wqkv_p.ap()[d * 128:(d + 1) * 128,
                                        ch * 512:(ch + 1) * 512].bitcast(F32R))
                    for t in range(NT):
                        nc.tensor.matmul(
                            ps[t], h_T[d][:, t * 128:(t + 1) * 128].bitcast(F32R),
                            w.bitcast(F32R), start=(d == 0), stop=(d == ND - 1))
                for t in range(NT):
                    if ch < 2:        # q chunks
                        dst = q_N[t][:, (ch % 2) * 512:(ch % 2) * 512 + 512]
                        src_bias = ("b_qkv", ch * 512)
                    elif ch < 4:      # k chunks
                        dst = k_N[t][:, (ch % 2) * 512:(ch % 2) * 512 + 512]
                        src_bias = ("b_qkv", ch * 512)
                    else:             # v chunks -> strided pad write
                        h0 = (ch - 4) * 8
                        dst = v_pad[t][:, h0:h0 + 8, 0:HD].bitcast(F32R)
                        if "b_qkv" in bc_tiles:
                            nc.vector.tensor_add(
                                out=dst,
                                in0=bc_tiles["b_qkv"][:, ch * 512:(ch + 1) * 512]
                                .rearrange("p (h f) -> p h f", h=8),
                                in1=ps[t].rearrange("p (h f) -> p h f", h=8))
                        else:
                            nc.vector.tensor_copy(
                                out=dst,
                                in_=ps[t].rearrange("p (h f) -> p h f", h=8))
                        continue
                    if "b_qkv" in bc_tiles:
                        nc.vector.tensor_add(
                            out=dst,
                            in0=bc_tiles["b_qkv"][:, src_bias[1]:src_bias[1] + 512],
                            in1=ps[t])
                    else:
                        nc.vector.tensor_copy(out=dst, in_=ps[t])

        # ---- Phase C: QK-norm + rope + transpose -------------------------

        def qknorm_stats(src_tile, gname):
            st = stat.tile([128, 2, 6], F32, tag="qkst")
            nc.vector.bn_stats(out=st[:, 0, :], in_=src_tile[:, 0:512])
            nc.vector.bn_stats(out=st[:, 1, :], in_=src_tile[:, 512:1024])
            mv = stat.tile([128, 2], F32, tag="qkmv")
            nc.vector.bn_aggr(out=mv, in_=st)
            rstd = stat.tile([128, 1], F32, tag="qkrstd")
            nc.scalar.activation(out=rstd, in_=mv[:, 1:2], func=AF.Sqrt,
                                 bias=eps_t, scale=1.0, alpha=0.0)
            nc.vector.reciprocal(out=rstd, in_=rstd)
            negmr = stat.tile([128, 1], F32, tag="qknm")
            nc.vector.tensor_mul(out=negmr, in0=mv[:, 0:1], in1=rstd)
            nc.scalar.mul(out=negmr, in_=negmr, mul=-1.0)
            return rstd, negmr

        with (
            tc.tile_pool(name="cspool", bufs=2) as cspool,
            tc.tile_pool(name="ropep", bufs=2) as ropep,
            tc.tile_pool(name="trps2", bufs=4, space="PSUM") as trps2,
        ):
            for t in range(NT):
                cosf = cspool.tile([128, D], F32, tag="cosf")
                sinm = cspool.tile([128, D], F32, tag="sinm")
                nc.sync.dma_start(out=cosf, in_=cos_p.ap()[t * 128:(t + 1) * 128, :])
                nc.sync.dma_start(out=sinm, in_=sin_p.ap()[t * 128:(t + 1) * 128, :])
                for which, src_N, dst_T, gname in (
                    ("q", q_N[t], q_T, "qn"), ("k", k_N[t], k_T, "kn"),
                ):
                    rstd, negmr = qknorm_stats(src_N, gname)
                    nrm = ropep.tile([128, D], F32, tag="nrm")
                    nc.scalar.activation(out=nrm, in_=src_N, func=AF.Identity,
                                         scale=rstd, bias=negmr, alpha=0.0)
                    if f"{gname}_g" in bc_tiles:
                        nc.vector.tensor_mul(out=nrm, in0=nrm,
                                             in1=bc_tiles[f"{gname}_g"])
                        nc.vector.tensor_add(out=nrm, in0=nrm,
                                             in1=bc_tiles[f"{gname}_b"])
                    nrm3 = nrm.rearrange("p (h f) -> p h f", h=H)
                    sw = ropep.tile([128, H, HD], F32, tag="sw")
                    nc.vector.tensor_copy(out=sw[:, :, 0:32], in_=nrm3[:, :, 32:64])
                    nc.vector.tensor_copy(out=sw[:, :, 32:64], in_=nrm3[:, :, 0:32])
                    swf = sw.rearrange("p h f -> p (h f)")
                    rp = ropep.tile([128, D], F32, tag="rp")
                    nc.vector.tensor_mul(out=rp, in0=nrm, in1=cosf)
                    nc.vector.tensor_mul(out=swf, in0=swf, in1=sinm)
                    nc.vector.tensor_add(out=rp, in0=rp, in1=swf)
                    for d in range(ND):
                        ptr = trps2.tile([128, 128], F32, tag="trp2")
                        nc.tensor.transpose(ptr, rp[:, d * 128:(d + 1) * 128], ident)
                        nc.vector.tensor_copy(
                            out=dst_T[d][:, t * 128:(t + 1) * 128].bitcast(F32R),
                            in_=ptr)

        hTpool_cm.__exit__(None, None, None)
        qknp_cm.__exit__(None, None, None)
        # ---- Phase D: bounce + grouped AllGather -------------------------
        for d in range(ND):
            dst = bass.AP(tensor=kv_in.ap().tensor, offset=d * 128 * T,
                          ap=[[T, 128], [1, T]])
            nc.sync.dma_start(out=dst.bitcast(F32R), in_=k_T[d].bitcast(F32R))
        voff = D * T
        for t in range(NT):
            dst = bass.AP(tensor=kv_in.ap().tensor, offset=voff + t * 128 * VW,
                          ap=[[VW, 128], [1, VW]])
            nc.sync.dma_start(out=dst.bitcast(F32R),
                              in_=v_pad[t].rearrange("p h f -> p (h f)").bitcast(F32R))
        groups = [list(range(g * GROUP, (g + 1) * GROUP))
                  for g in range(N_CORES // GROUP)]
        nc.gpsimd.collective_compute(
            "AllGather", mybir.AluOpType.bypass, replica_groups=groups,
            ins=[kv_in.ap().opt()], outs=[kv_all.ap().opt()])

        # ---- Phase E: attention ------------------------------------------
        attp_cm = tc.tile_pool(name="attp", bufs=1)
        attp = attp_cm.__enter__()
        accA = [attp.tile([65, T], F32, tag=f"accA{d}", name=f"accA{d}") for d in range(HP)]
        accB = [attp.tile([65, T], F32, tag=f"accB{d}", name=f"accB{d}") for d in range(HP)]
        stacked = [attp.tile([128, T], F32, tag=f"stk{d}", name=f"stk{d}") for d in range(HP)]

        KT_HALF = (GROUP * T // 128) // 2   # ktok tiles per half (8 full-size)
        CH_HALF = GROUP // 2                # rank chunks per half

        for ha in range(2):
            with (
                tc.tile_pool(name=f"vh{ha}", bufs=1) as vh,
                tc.tile_pool(name=f"kh{ha}", bufs=3) as kh,
                tc.tile_pool(name=f"scps{ha}", bufs=2, space="PSUM") as scps,
                tc.tile_pool(name=f"pvps{ha}", bufs=1, space="PSUM") as pvps,
                tc.tile_pool(name=f"prb{ha}", bufs=4) as prb,
            ):
                vtiles = []
                for i in range(KT_HALF):
                    rc = ha * CH_HALF + i // (T // 128)
                    st = i % (T // 128)
                    vt = vh.tile([128, VW], F32, tag=f"vt{i}", name=f"vt{i}_{ha}")
                    src = bass.AP(tensor=kv_all.ap().tensor,
                                  offset=rc * KVF + voff + st * 128 * VW,
                                  ap=[[VW, 128], [1, VW]])
                    nc.sync.dma_start(out=vt.bitcast(F32R), in_=src.bitcast(F32R))
                    vtiles.append(vt)
                for d in range(HP):
                    ks = []
                    for c2 in range(CH_HALF):
                        rc = ha * CH_HALF + c2
                        kt_ = kh.tile([128, T], F32, tag="kt")
                        src = bass.AP(tensor=kv_all.ap().tensor,
                                      offset=rc * KVF + d * 128 * T,
                                      ap=[[T, 128], [1, T]])
                        nc.sync.dma_start(out=kt_.bitcast(F32R),
                                          in_=src.bitcast(F32R))
                        ks.append(kt_)
                    pvA = pvps.tile([65, T], F32, tag="pvA")
                    pvB = pvps.tile([65, T], F32, tag="pvB")
                    hA, hB = 2 * d, 2 * d + 1
                    for kt in range(KT_HALF):
                        c2, st = divmod(kt, T // 128)
                        sl = slice(st * 128, (st + 1) * 128)
                        psA = scps.tile([128, T], F32, tag="psA")
                        psB = scps.tile([128, T], F32, tag="psB")
                        nc.tensor.matmul(psA, ks[c2][0:64, sl].bitcast(F32R),
                                         q_T[d][0:64, :].bitcast(F32R),
                                         start=True, stop=True,
                                         tile_position=(0, 0))
                        nc.tensor.matmul(psB, ks[c2][64:128, sl].bitcast(F32R),
                                         q_T[d][64:128, :].bitcast(F32R),
                                         start=True, stop=True,
                                         tile_position=(64, 0))
                        prA = prb.tile([128, T], F32, tag="prA")
                        prB = prb.tile([128, T], F32, tag="prB")
                        nc.scalar.activation(out=prA.bitcast(F32R), in_=psA,
                                             func=AF.Exp, scale=1.0 / np.sqrt(HD),
                                             alpha=0.0)
                        nc.scalar.activation(out=prB.bitcast(F32R), in_=psB,
                                             func=AF.Exp, scale=1.0 / np.sqrt(HD),
                                             alpha=0.0)
                        vt = vtiles[kt]
                        v3 = vt.rearrange("p (h f) -> p h f", h=H)
                        nc.tensor.matmul(pvA, v3[:, hA, :].bitcast(F32R),
                                         prA.bitcast(F32R),
                                         start=(kt == 0), stop=(kt == KT_HALF - 1))
                        nc.tensor.matmul(pvB, v3[:, hB, :].bitcast(F32R),
                                         prB.bitcast(F32R),
                                         start=(kt == 0), stop=(kt == KT_HALF - 1))
                    if ha == 0:
                        nc.vector.tensor_copy(out=accA[d].bitcast(F32R), in_=pvA)
                        nc.vector.tensor_copy(out=accB[d].bitcast(F32R), in_=pvB)
                    else:
                        nc.vector.tensor_add(out=accA[d].bitcast(F32R),
                                             in0=accA[d], in1=pvA)
                        nc.vector.tensor_add(out=accB[d].bitcast(F32R),
                                             in0=accB[d], in1=pvB)

        # scale by 1/denominator and restack head pairs
        with (
            tc.tile_pool(name="bcps", bufs=2, space="PSUM") as bcps,
            tc.tile_pool(name="tbp", bufs=2) as tbp,
        ):
            for d in range(HP):
                with nc.allow_low_precision(reason="f32r bits are f32"):
                    nc.vector.reciprocal(out=accA[d][64:65, :].bitcast(F32R),
                                         in_=accA[d][64:65, :])
                    nc.vector.reciprocal(out=accB[d][64:65, :].bitcast(F32R),
                                         in_=accB[d][64:65, :])
                bcA = bcps.tile([128, T], F32, tag="bcA")
                nc.tensor.matmul(bcA, sel.bitcast(F32R), accA[d].bitcast(F32R),
                                 start=True, stop=True)
                nc.vector.tensor_mul(out=stacked[d][0:64, :].bitcast(F32R),
                                     in0=accA[d][0:64, :], in1=bcA[0:64, :])
                bcB = bcps.tile([128, T], F32, tag="bcB")
                nc.tensor.matmul(bcB, sel.bitcast(F32R), accB[d].bitcast(F32R),
                                 start=True, stop=True)
                tmpB = tbp.tile([64, T], F32, tag="tmpB")
                nc.vector.tensor_mul(out=tmpB.bitcast(F32R),
                                     in0=accB[d][0:64, :], in1=bcB[0:64, :])
                nc.sync.dma_start(out=stacked[d][64:128, :].bitcast(F32R),
                                  in_=tmpB.bitcast(F32R))

        # ---- Phase F: out projection + residual --------------------------
        with (
            tc.tile_pool(name="wo", bufs=3) as wo,
            tc.tile_pool(name="ops", bufs=8, space="PSUM") as ops,
        ):
            for ch in range(D // 512):
                ps = [ops.tile([128, 512], F32, tag="ops", name=f"ops_{ch}_{t}") for t in range(NT)]
                for d in range(HP):
                    w = wo.tile([128, 512], F32, tag="wot")
                    nc.sync.dma_start(
                        out=w.bitcast(F32R),
                        in_=wout_p.ap()[d * 128:(d + 1) * 128,
                                        ch * 512:(ch + 1) * 512].bitcast(F32R))
                    for t in range(NT):
                        nc.tensor.matmul(
                            ps[t], stacked[d][:, t * 128:(t + 1) * 128].bitcast(F32R),
                            w.bitcast(F32R), start=(d == 0), stop=(d == HP - 1))
                for t in range(NT):
                    sl = slice(ch * 512, (ch + 1) * 512)
                    nc.vector.tensor_add(out=out1_N[t][:, sl],
                                         in0=x_N[t][:, sl], in1=ps[t])
                    if "b_out" in bc_tiles:
                        nc.vector.tensor_add(out=out1_N[t][:, sl],
                                             in0=out1_N[t][:, sl],
                                             in1=bc_tiles["b_out"][:, sl])

        attp_cm.__exit__(None, None, None)
        qkv_res_cm.__exit__(None, None, None)
        # ---- Phase G: LN2 + transpose ------------------------------------
        h2p = stack.enter_context(tc.tile_pool(name="h2p", bufs=1))
        h2_T = [h2p.tile([128, T], F32, tag=f"h2T{d}", name=f"h2T{d}") for d in range(ND)]
        with (
            tc.tile_pool(name="h2pool", bufs=2) as h2pool,
            tc.tile_pool(name="trps3", bufs=4, space="PSUM") as trps3,
        ):
            for t in range(NT):
                h2_N = h2pool.tile([128, D], F32, tag="h2N")
                layer_norm_tiles(out1_N[t], h2_N, "ln2")
                for d in range(ND):
                    ptr = trps3.tile([128, 128], F32, tag="trp3")
                    nc.tensor.transpose(ptr, h2_N[:, d * 128:(d + 1) * 128], ident)
                    nc.vector.tensor_copy(
                        out=h2_T[d][:, t * 128:(t + 1) * 128].bitcast(F32R), in_=ptr)

        # ---- Phase H: FFN -------------------------------------------------
        prp = stack.enter_context(tc.tile_pool(name="prp", bufs=1))
        prod_T = [prp.tile([128, T], F32, tag=f"pr{h}", name=f"pr{h}") for h in range(NH)]
        with (
            tc.tile_pool(name="wf", bufs=3) as wf,
            tc.tile_pool(name="ffps", bufs=2, space="PSUM") as ffps,
            tc.tile_pool(name="s1p", bufs=2) as s1p,
        ):
            for ht in range(NH):
                w1sb = wf.tile([128, ND, 128], F32, tag="w1sb")
                w3sb = wf.tile([128, ND, 128], F32, tag="w3sb")
                for wsb, wp in ((w1sb, w1_p), (w3sb, w3_p)):
                    src = bass.AP(tensor=wp.ap().tensor, offset=ht * 128,
                                  ap=[[FFN, 128], [128 * FFN, ND], [1, 128]])
                    nc.sync.dma_start(out=wsb.bitcast(F32R), in_=src.bitcast(F32R))
                ps1 = ffps.tile([128, T], F32, tag="ps1")
                ps3 = ffps.tile([128, T], F32, tag="ps3")
                for d in range(ND):
                    nc.tensor.matmul(ps1, w1sb[:, d, :].bitcast(F32R),
                                     h2_T[d].bitcast(F32R),
                                     start=(d == 0), stop=(d == ND - 1))
                for d in range(ND):
                    nc.tensor.matmul(ps3, w3sb[:, d, :].bitcast(F32R),
                                     h2_T[d].bitcast(F32R),
                                     start=(d == 0), stop=(d == ND - 1))
                s1 = s1p.tile([128, T], F32, tag="s1")
                b1arg = bc_tiles["b1"][:, ht:ht + 1] if "b1" in bc_tiles else 0.0
                nc.scalar.activation(out=s1, in_=ps1, func=AF.Silu,
                                     bias=b1arg, scale=1.0, alpha=0.0)
                if "b3" in bc_tiles:
                    t3 = s1p.tile([128, T], F32, tag="t3")
                    nc.vector.tensor_scalar_add(
                        out=t3, in0=ps3, scalar1=bc_tiles["b3"][:, ht:ht + 1])
                    nc.vector.tensor_mul(out=prod_T[ht].bitcast(F32R),
                                         in0=s1, in1=t3)
                else:
                    nc.vector.tensor_mul(out=prod_T[ht].bitcast(F32R),
                                         in0=s1, in1=ps3)

        with (
            tc.tile_pool(name="w2p", bufs=3) as w2p,
            tc.tile_pool(name="f2ps", bufs=8, space="PSUM") as f2ps,
            tc.tile_pool(name="finp", bufs=2) as finp,
        ):
            for ch in range(D // 512):
                ps = [f2ps.tile([128, 512], F32, tag="f2", name=f"f2_{ch}_{t}") for t in range(NT)]
                for ht in range(NH):
                    w = w2p.tile([128, 512], F32, tag="w2t")
                    nc.sync.dma_start(
                        out=w.bitcast(F32R),
                        in_=w2_p.ap()[ht * 128:(ht + 1) * 128,
                                      ch * 512:(ch + 1) * 512].bitcast(F32R))
                    for t in range(NT):
                        nc.tensor.matmul(
                            ps[t], prod_T[ht][:, t * 128:(t + 1) * 128].bitcast(F32R),
                            w.bitcast(F32R), start=(ht == 0), stop=(ht == NH - 1))
                for t in range(NT):
                    sl = slice(ch * 512, (ch + 1) * 512)
                    fin = finp.tile([128, 512], F32, tag="fin")
                    nc.vector.tensor_add(out=fin, in0=out1_N[t][:, sl], in1=ps[t])
                    if "b2" in bc_tiles:
                        nc.vector.tensor_add(out=fin, in0=fin,
                                             in1=bc_tiles["b2"][:, sl])
                    nc.sync.dma_start(out=out_p.ap()[t * 128:(t + 1) * 128, sl],
                                      in_=fin)

    _split_all_waits(nc)
    return nc


# ---------------------------------------------------------------------------
# Host wrapper
# ---------------------------------------------------------------------------

_CACHE = {}


def _prep_inputs(x, rope_cos, rope_sin, w_qkv, b_qkv, w_out, b_out,
                 qn_g, qn_b, kn_g, kn_b, ln1_g, ln1_b, ln2_g, ln2_b,
                 w1, b1, w2, b2, w3, b3):
    B, S, D = x.shape
    H, HD = 16, 64
    T = B * S // N_CORES

    flags = set()
    if not (np.all(ln1_g == 1) and np.all(ln1_b == 0)):
        flags.add("ln1_gb")
    if not (np.all(qn_g == 1) and np.all(qn_b == 0)):
        flags.add("qn_gb")
    if not (np.all(kn_g == 1) and np.all(kn_b == 0)):
        flags.add("kn_gb")
    if not (np.all(ln2_g == 1) and np.all(ln2_b == 0)):
        flags.add("ln2_gb")
    if np.any(b_qkv != 0):
        flags.add("bqkv")
    if np.any(b_out != 0):
        flags.add("bout")
    if np.any(b1 != 0):
        flags.add("b1")
    if np.any(b2 != 0):
        flags.add("b2")
    if np.any(b3 != 0):
        flags.add("b3")

    # host-side rope tables: [S, D] tiled over heads, rotation sign folded in
    cosfull = np.tile(rope_cos, (1, H)).astype(np.float32)          # [S, D]
    sinmod_half = np.concatenate(
        [-rope_sin[:, :HD // 2], rope_sin[:, HD // 2:]], axis=1)    # [S, HD]
    sinmod = np.tile(sinmod_half, (1, H)).astype(np.float32)        # [S, D]

    wqkvT = np.ascontiguousarray(w_qkv.T).astype(np.float32)
    woutT = np.ascontiguousarray(w_out.T).astype(np.float32)
    w1T = np.ascontiguousarray(w1.T).astype(np.float32)
    w3T = np.ascontiguousarray(w3.T).astype(np.float32)
    w2T = np.ascontiguousarray(w2.T).astype(np.float32)

    xf = np.ascontiguousarray(x.reshape(B * S, D)).astype(np.float32)
    in_maps = []
    for c in range(N_CORES):
        t0 = c * T
        m = {
            "x": xf[t0:t0 + T],
            "cosfull": np.ascontiguousarray(cosfull[t0 % S:t0 % S + T]),
            "sinmod": np.ascontiguousarray(sinmod[t0 % S:t0 % S + T]),
            "wqkvT": wqkvT, "woutT": woutT,
            "w1T": w1T, "w3T": w3T, "w2T": w2T,
        }
        opt = {"ln1_gb": [("ln1_g", ln1_g), ("ln1_b", ln1_b)],
               "qn_gb": [("qn_g", qn_g), ("qn_b", qn_b)],
               "kn_gb": [("kn_g", kn_g), ("kn_b", kn_b)],
               "ln2_gb": [("ln2_g", ln2_g), ("ln2_b", ln2_b)],
               "bqkv": [("b_qkv", b_qkv)], "bout": [("b_out", b_out)],
               "b1": [("b1", b1)], "b2": [("b2", b2)], "b3": [("b3", b3)]}
        for fl, items in opt.items():
            if fl in flags:
                for name, arr in items:
                    m[name] = np.ascontiguousarray(arr).astype(np.float32)
        in_maps.append(m)
    return in_maps, frozenset(flags), T, D


def kernel(**inputs):
    x = inputs["x"]
    B, S, D = x.shape
    in_maps, flags, T, _ = _prep_inputs(**inputs)

    key = (T, D, flags)
    if key not in _CACHE:
        _CACHE[key] = build_nc(T=T, D=D, flags=flags)
    nc = _CACHE[key]

    res = run_bass_kernel_spmd(nc, in_maps, core_ids=list(range(N_CORES)))
    out = np.empty((B * S, D), np.float32)
    for c in range(N_CORES):
        out[c * T:(c + 1) * T] = res.results[c]["out"]
    return out.reshape(B, S, D)
